# revision 9
# baseline (speedup 1.0000x reference)
"""NeuroSAT message-passing RNN on 8 Trainium2 NeuronCores.

Strategy (per core, 8 graphs: 4096 literals, 8192 clauses):
- All states resident in SBUF, TRANSPOSED layout [D=128 partitions, nodes].
- The sparse clause<->literal segment-sums are computed on the TensorEngine as
  block-dense one-hot (count) matmuls against per-graph adjacency blocks in
  bf16, fused with the RNN input projections:
    LC:  pre_c = (W_ih_lc @ x_l^T) @ A^T + W_hh_lc @ x_c^T
         via y_l = x_l @ W_ih_lc^T (normal layout), then y_l-as-weights @ A_gT
    CL:  pre_l = (W_A @ x_cnew^T) @ A + W_B @ flip(x_l)^T + W_hh_cl @ x_l^T
         via z_c = x_cnew @ W_A^T (normal layout), then z_c-as-weights @ A_g
- tanh+bias on ScalarE from PSUM; per-column l2 norms via ones-matmul
  (float32r) + reciprocal/sqrt + outer-product broadcast matmul; states kept
  bf16; fp32 (unnormalized) clause snapshots + per-column inv norms are DMA'd
  out and normalized on the host.
"""

import numpy as np
import ml_dtypes

import concourse.bass as bass
import concourse.tile as tile
from concourse import bacc, mybir
from concourse.bass import ts
from concourse.bass_utils import run_bass_kernel_spmd

F32 = mybir.dt.float32
F32R = mybir.dt.float32r
BF16 = mybir.dt.bfloat16
FP8 = mybir.dt.float8e4
AF = mybir.ActivationFunctionType

B = 64
LITS_PER = 512
CLS_PER = 1024
NL = B * LITS_PER
NC = B * CLS_PER
D = 128
T = 8
M = 8                    # cores
GPC = B // M             # graphs per core
NLc = GPC * LITS_PER     # 4096
NCc = GPC * CLS_PER      # 8192

_CACHE = {}
OMIT = set()   # bisection flags: "y", "cls", "lit", "norm", "dma"
PROFILE = False
LAST_RESULTS = None


def _build_kernel():
    nc = bacc.Bacc(None, target_bir_lowering=False)

    # ---- DRAM inputs ----
    xl0T = nc.dram_tensor("xl0T", [D, NLc], BF16, kind="ExternalInput")
    xc0T = nc.dram_tensor("xc0T", [D, NCc], BF16, kind="ExternalInput")
    adj_lc = nc.dram_tensor("adj_lc", [D, GPC * 4 * CLS_PER], FP8, kind="ExternalInput")
    adj_cl = nc.dram_tensor("adj_cl", [D, GPC * 8 * LITS_PER], FP8, kind="ExternalInput")
    w_yl = nc.dram_tensor("w_yl", [D, D], BF16, kind="ExternalInput")       # W_ih_lc^T (rhs)
    whh_lc_T = nc.dram_tensor("whh_lc_T", [D, D], BF16, kind="ExternalInput")
    wa_T = nc.dram_tensor("wa_T", [D, D], BF16, kind="ExternalInput")       # W_ih_cl[:, :D]^T (rhs)
    wb_T = nc.dram_tensor("wb_T", [D, D], BF16, kind="ExternalInput")       # (W_ih_cl[:, D:])^T (lhsT)
    whh_cl_T = nc.dram_tensor("whh_cl_T", [D, D], BF16, kind="ExternalInput")
    bias_lc_in = nc.dram_tensor("bias_lc", [D, 1], F32, kind="ExternalInput")
    bias_cl_in = nc.dram_tensor("bias_cl", [D, 1], F32, kind="ExternalInput")
    ones_c_in = nc.dram_tensor("ones_c", [D, 1], F32R, kind="ExternalInput")
    ones_r_in = nc.dram_tensor("ones_r", [1, D], F32R, kind="ExternalInput")
    tv_in = nc.dram_tensor("tv", [D, 1], BF16, kind="ExternalInput")        # true_vec_w^T (lhsT)

    # ---- DRAM outputs ----
    clause_out = nc.dram_tensor("clause_out", [T, D, NCc], F32, kind="ExternalOutput")
    inv_c_out = nc.dram_tensor("inv_c_out", [T, NCc], F32R, kind="ExternalOutput")
    truth_out = nc.dram_tensor("truth_out", [T, NLc], F32, kind="ExternalOutput")
    xl_fin_out = nc.dram_tensor("xl_fin_out", [D, NLc], F32, kind="ExternalOutput")

    with tile.TileContext(nc) as tc:
        with (
            tc.tile_pool(name="cst", bufs=1) as cst,
            tc.tile_pool(name="st", bufs=1) as st,
            tc.tile_pool(name="wk", bufs=3) as wk,
            tc.tile_pool(name="nrm", bufs=3) as nrm,
            tc.tile_pool(name="pmm", bufs=2, space="PSUM") as pmm,
            tc.tile_pool(name="pbc", bufs=2, space="PSUM") as pbc,
            tc.tile_pool(name="psm", bufs=2, space="PSUM") as psm,
            tc.tile_pool(name="pnr", bufs=2, space="PSUM") as pnr,
        ):
            # persistent tensors
            xlT = st.tile([D, NLc], BF16, tag="xlT")
            xcT = st.tile([D, NCc], BF16, tag="xcT")
            y_l = st.tile([D, NLc], BF16, tag="y_l")       # 32 normal-layout tiles
            z_c = st.tile([D, NCc], BF16, tag="z_c")       # 64 normal-layout tiles
            a_lc = cst.tile([D, GPC * 4 * CLS_PER], FP8, tag="a_lc")
            a_cl = cst.tile([D, GPC * 8 * LITS_PER], FP8, tag="a_cl")
            wyl = cst.tile([D, D], BF16, tag="wyl")
            whhlc = cst.tile([D, D], BF16, tag="whhlc")
            waT = cst.tile([D, D], BF16, tag="waT")
            wbT = cst.tile([D, D], BF16, tag="wbT")
            whhcl = cst.tile([D, D], BF16, tag="whhcl")
            b_lc = cst.tile([D, 1], F32, tag="b_lc")
            b_cl = cst.tile([D, 1], F32, tag="b_cl")
            ones_c = cst.tile([D, 1], F32R, tag="ones_c")
            ones_r = cst.tile([1, D], F32R, tag="ones_r")
            tv = cst.tile([D, 1], BF16, tag="tv")

            nc.gpsimd.dma_start(xlT[:], xl0T[:])
            nc.gpsimd.dma_start(xcT[:], xc0T[:])
            nc.gpsimd.dma_start(a_lc[:], adj_lc[:])
            nc.gpsimd.dma_start(a_cl[:], adj_cl[:])
            nc.gpsimd.dma_start(wyl[:], w_yl[:])
            nc.gpsimd.dma_start(whhlc[:], whh_lc_T[:])
            nc.gpsimd.dma_start(waT[:], wa_T[:])
            nc.gpsimd.dma_start(wbT[:], wb_T[:])
            nc.gpsimd.dma_start(whhcl[:], whh_cl_T[:])
            nc.gpsimd.dma_start(b_lc[:], bias_lc_in[:])
            nc.gpsimd.dma_start(b_cl[:], bias_cl_in[:])
            nc.gpsimd.dma_start(ones_c[:], ones_c_in[:])
            nc.gpsimd.dma_start(ones_r[:], ones_r_in[:])
            nc.gpsimd.dma_start(tv[:], tv_in[:])

            def col_norm_inv(stage, n_cols):
                """Return ([128, n_cols] PSUM bcast of inv l2 norm, inv tile)."""
                sq = wk.tile([D, n_cols], F32R, tag="sq")
                nc.gpsimd.tensor_tensor(sq[:], stage[:], stage[:], op=mybir.AluOpType.mult)
                psn = pnr.tile([1, n_cols], F32, tag="nrm")
                nc.tensor.matmul(psn[:], ones_c[:], sq[:], start=True, stop=True)
                rec = nrm.tile([1, n_cols], F32, tag="rec")
                nc.vector.reciprocal(rec[:], psn[:])
                inv = nrm.tile([1, n_cols], F32R, tag="inv")
                nc.scalar.activation(inv[:], rec[:], AF.Sqrt)
                psb = pbc.tile([D, n_cols], F32, tag="bc")
                nc.tensor.matmul(psb[:], ones_r[:], inv[:], start=True, stop=True)
                return psb, inv

            for t in range(T):
                # ---- y_l = x_l @ W_ih_lc^T (normal layout, 32 tiles) ----
                for j in range(NLc // D if "y" not in OMIT else 0):
                    psy = psm.tile([D, D], F32, tag="zy")
                    nc.tensor.matmul(psy[:], xlT[:, ts(j, D)], wyl[:], start=True, stop=True)
                    nc.scalar.activation(y_l[:, ts(j, D)], psy[:], AF.Copy)

                # ---- clause update ----
                for n in range(NCc // 512 if "cls" not in OMIT else 0):
                    g, half = divmod(n, 2)
                    ps = pmm.tile([D, 512], F32, tag="mm")
                    for kt in range(4):
                        nc.tensor.matmul(
                            ps[:],
                            y_l[:, ts(g * 4 + kt, D)],
                            a_lc[:, g * 4 * CLS_PER + kt * CLS_PER + half * 512:][:, :512],
                            start=(kt == 0),
                            stop=False,
                        )
                    nc.tensor.matmul(ps[:], whhlc[:], xcT[:, ts(n, 512)], start=False, stop=True)
                    stage = wk.tile([D, 512], F32, tag="stage")
                    nc.scalar.activation(stage[:], ps[:], AF.Tanh, bias=b_lc[:])
                    nc.sync.dma_start(clause_out[t, :, ts(n, 512)], stage[:])
                    psb, inv = col_norm_inv(stage, 512)
                    nc.sync.dma_start(inv_c_out[t, ts(n, 512)], inv[:])
                    nc.vector.tensor_mul(xcT[:, ts(n, 512)], stage[:], psb[:])
                    # z_c = x_cnew @ W_A^T (normal layout)
                    xcb = wk.tile([D, 512], BF16, tag="xcb")
                    nc.vector.tensor_copy(xcb[:], stage[:])
                    for q in range(4):
                        psz = psm.tile([D, D], F32, tag="zy")
                        nc.tensor.matmul(psz[:], xcb[:, ts(q, D)], waT[:], start=True, stop=True)
                        nc.scalar.activation(z_c[:, ts(n * 4 + q, D)], psz[:], AF.Copy)

                # ---- literal update ----
                for g in range(GPC if "lit" not in OMIT else 0):
                    ps = pmm.tile([D, 512], F32, tag="mm")
                    for kt in range(8):
                        nc.tensor.matmul(
                            ps[:],
                            z_c[:, ts(g * 8 + kt, D)],
                            a_cl[:, g * 8 * LITS_PER + kt * LITS_PER:][:, :512],
                            start=(kt == 0),
                            stop=False,
                        )
                    o = g * LITS_PER
                    if "flip" not in OMIT:
                        nc.tensor.matmul(ps[:, 0:256], wbT[:], xlT[:, o + 256:o + 512], start=False, stop=False)
                        nc.tensor.matmul(ps[:, 256:512], wbT[:], xlT[:, o:o + 256], start=False, stop=False)
                    nc.tensor.matmul(ps[:], whhcl[:], xlT[:, ts(g, 512)], start=False, stop=True)
                    stage = wk.tile([D, 512], F32, tag="stage")
                    nc.scalar.activation(stage[:], ps[:], AF.Tanh, bias=b_cl[:])
                    psb, _inv = col_norm_inv(stage, 512)
                    nc.vector.tensor_mul(xlT[:, ts(g, 512)], stage[:], psb[:])
                    if t == T - 1:
                        xlf = wk.tile([D, 512], F32, tag="xlf")
                        nc.vector.tensor_mul(xlf[:], stage[:], psb[:])
                        nc.sync.dma_start(xl_fin_out[:, ts(g, 512)], xlf[:])
                    if "truth" not in OMIT:
                        pst = pnr.tile([1, 512], F32, tag="nrm")
                        nc.tensor.matmul(pst[:], tv[:], xlT[:, ts(g, 512)], start=True, stop=True)
                        trs = nrm.tile([1, 512], F32, tag="trs")
                        nc.scalar.activation(trs[:], pst[:], AF.Copy)
                        if "truthdma" not in OMIT:
                            nc.sync.dma_start(truth_out[t:t + 1, ts(g, 512)], trs[:1, :])

    nc.finalize()
    return nc


def _prep_inputs(edge_clause, edge_lit, x_l0, x_c0,
                 W_ih_lc, W_hh_lc, b_ih_lc, b_hh_lc,
                 W_ih_cl, W_hh_cl, b_ih_cl, b_hh_cl, true_vec_w):
    bf = ml_dtypes.bfloat16
    xl_n = x_l0 / np.linalg.norm(x_l0, axis=1, keepdims=True)
    xc_n = x_c0 / np.linalg.norm(x_c0, axis=1, keepdims=True)

    lit_local = (edge_lit % LITS_PER).astype(np.int64)
    A = np.zeros((NC, LITS_PER), np.float32)
    np.add.at(A, (edge_clause.astype(np.int64), lit_local), 1.0)

    in_maps = []
    shared = {
        "w_yl": W_ih_lc.T.copy().astype(bf),
        "whh_lc_T": W_hh_lc.T.copy().astype(bf),
        "wa_T": W_ih_cl[:, :D].T.copy().astype(bf),
        "wb_T": W_ih_cl[:, D:].T.copy().astype(bf),
        "whh_cl_T": W_hh_cl.T.copy().astype(bf),
        "bias_lc": (b_ih_lc + b_hh_lc).reshape(D, 1).astype(np.float32),
        "bias_cl": (b_ih_cl + b_hh_cl).reshape(D, 1).astype(np.float32),
        "ones_c": np.ones((D, 1), np.float32),
        "ones_r": np.ones((1, D), np.float32),
        "tv": true_vec_w.reshape(1, D).T.copy().astype(bf),
    }
    for m in range(M):
        lit0, cl0 = m * NLc, m * NCc
        xlT = np.ascontiguousarray(xl_n[lit0:lit0 + NLc].T).astype(bf)
        xcT = np.ascontiguousarray(xc_n[cl0:cl0 + NCc].T).astype(bf)
        # adj_lc: per graph g, kt in 0..3: A_g.T[kt*128:(kt+1)*128, :]  [128, 1024]
        a_lc = np.empty((D, GPC * 4 * CLS_PER), np.float32)
        a_cl = np.empty((D, GPC * 8 * LITS_PER), np.float32)
        for g in range(GPC):
            Ag = A[cl0 + g * CLS_PER: cl0 + (g + 1) * CLS_PER]       # [1024, 512]
            AgT = Ag.T                                                # [512, 1024]
            for kt in range(4):
                a_lc[:, (g * 4 + kt) * CLS_PER:(g * 4 + kt + 1) * CLS_PER] = \
                    AgT[kt * D:(kt + 1) * D]
            for kt in range(8):
                a_cl[:, (g * 8 + kt) * LITS_PER:(g * 8 + kt + 1) * LITS_PER] = \
                    Ag[kt * D:(kt + 1) * D]
        in_maps.append({
            "xl0T": xlT, "xc0T": xcT,
            "adj_lc": a_lc.astype(ml_dtypes.float8_e4m3fn),
            "adj_cl": a_cl.astype(ml_dtypes.float8_e4m3fn),
            **shared,
        })
    return in_maps, xl_n, xc_n


def kernel(edge_clause, edge_lit, x_l0, x_c0,
           W_ih_lc, W_hh_lc, b_ih_lc, b_hh_lc,
           W_ih_cl, W_hh_cl, b_ih_cl, b_hh_cl,
           L_vote_w, L_vote_b, true_vec_w, num_iters, **kw):
    edge_clause = np.asarray(edge_clause)
    edge_lit = np.asarray(edge_lit)
    x_l0 = np.asarray(x_l0, np.float32)
    x_c0 = np.asarray(x_c0, np.float32)
    W_ih_lc = np.asarray(W_ih_lc, np.float32)
    W_hh_lc = np.asarray(W_hh_lc, np.float32)
    b_ih_lc = np.asarray(b_ih_lc, np.float32)
    b_hh_lc = np.asarray(b_hh_lc, np.float32)
    W_ih_cl = np.asarray(W_ih_cl, np.float32)
    W_hh_cl = np.asarray(W_hh_cl, np.float32)
    b_ih_cl = np.asarray(b_ih_cl, np.float32)
    b_hh_cl = np.asarray(b_hh_cl, np.float32)
    L_vote_w = np.asarray(L_vote_w, np.float32)
    L_vote_b = np.asarray(L_vote_b, np.float32)
    true_vec_w = np.asarray(true_vec_w, np.float32)
    assert int(np.asarray(num_iters)) == T

    in_maps, xl_n0, xc_n0 = _prep_inputs(
        edge_clause, edge_lit, x_l0, x_c0, W_ih_lc, W_hh_lc, b_ih_lc, b_hh_lc,
        W_ih_cl, W_hh_cl, b_ih_cl, b_hh_cl, true_vec_w)

    if "nc" not in _CACHE:
        _CACHE["nc"] = _build_kernel()
    res = run_bass_kernel_spmd(_CACHE["nc"], in_maps, core_ids=list(range(M)),
                               trace=PROFILE)
    global LAST_RESULTS
    LAST_RESULTS = res

    # ---- host-side assembly ----
    truth_all = np.empty((T + 1, NL, 1), np.float32)
    clause_all = np.empty((T + 1, NC, D), np.float32)
    x_l = np.empty((NL, D), np.float32)
    truth_all[0] = (xl_n0 @ true_vec_w.T).reshape(NL, 1)
    clause_all[0] = xc_n0
    for m in range(M):
        r = res.results[m]
        lit0, cl0 = m * NLc, m * NCc
        truth_all[1:, lit0:lit0 + NLc, 0] = r["truth_out"]
        # clause snapshots: unnormalized [T, 128, NCc] * inv [T, NCc]
        snap = r["clause_out"] * r["inv_c_out"][:, None, :]
        clause_all[1:, cl0:cl0 + NCc, :] = snap.transpose(0, 2, 1)
        x_l[lit0:lit0 + NLc] = r["xl_fin_out"].T

    x_l_vote = x_l @ L_vote_w.T + L_vote_b
    vote_mean_pool = x_l_vote.reshape(B, LITS_PER, 1).mean(axis=1)
    return (x_l_vote, x_l, vote_mean_pool, truth_all[-1], truth_all,
            clause_all, truth_all[0])


# revision 10
# speedup vs baseline: 1.1949x; 1.1949x over previous
"""NeuroSAT message-passing RNN on 8 Trainium2 NeuronCores.

Strategy (per core, 8 graphs: 4096 literals, 8192 clauses):
- All states resident in SBUF, TRANSPOSED layout [D=128 partitions, nodes].
- The sparse clause<->literal segment-sums are computed on the TensorEngine as
  block-dense one-hot (count) matmuls against per-graph adjacency blocks in
  fp8 (counts are exact), fused with the RNN input projections:
    LC:  pre_c = (W_ih_lc @ x_l^T) @ A^T + W_hh_lc @ x_c^T
         via y_l = x_l @ W_ih_lc^T (normal layout), then y_l-as-weights @ A_gT
    CL:  pre_l = (W_A @ x_cnew^T) @ A + W_B @ flip(x_l)^T + W_hh_cl @ x_l^T
         via z_c = x_cnew @ W_A^T (normal layout), then z_c-as-weights @ A_g
- tanh+bias on ScalarE from PSUM; per-column l2 norms via an all-ones [128,128]
  matmul (f32r) that directly produces the partition-broadcast sum of squares,
  then DVE reciprocal + ACT sqrt (phase-grouped so the ACT function table is
  not thrashed between tanh and sqrt); states kept bf16; fp32 unnormalized
  clause snapshots + per-column inv norms are DMA'd out, normalized on host.
"""

import numpy as np
import ml_dtypes

import concourse.bass as bass
import concourse.tile as tile
from concourse import bacc, mybir
from concourse.bass import ts
from concourse.bass_utils import run_bass_kernel_spmd

F32 = mybir.dt.float32
F32R = mybir.dt.float32r
BF16 = mybir.dt.bfloat16
FP8 = mybir.dt.float8e4
AF = mybir.ActivationFunctionType

B = 64
LITS_PER = 512
CLS_PER = 1024
NL = B * LITS_PER
NC = B * CLS_PER
D = 128
T = 8
M = 8                    # cores
GPC = B // M             # graphs per core
NLc = GPC * LITS_PER     # 4096
NCc = GPC * CLS_PER      # 8192

_CACHE = {}
PROFILE = False
LAST_RESULTS = None


def _build_kernel():
    nc = bacc.Bacc(None, target_bir_lowering=False)

    # ---- DRAM inputs ----
    xl0T = nc.dram_tensor("xl0T", [D, NLc], BF16, kind="ExternalInput")
    xc0T = nc.dram_tensor("xc0T", [D, NCc], BF16, kind="ExternalInput")
    adj_lc = nc.dram_tensor("adj_lc", [D, GPC * 4 * CLS_PER], FP8, kind="ExternalInput")
    adj_cl = nc.dram_tensor("adj_cl", [D, GPC * 8 * LITS_PER], FP8, kind="ExternalInput")
    w_yl = nc.dram_tensor("w_yl", [D, D], BF16, kind="ExternalInput")       # W_ih_lc^T (rhs)
    whh_lc_T = nc.dram_tensor("whh_lc_T", [D, D], BF16, kind="ExternalInput")
    wa_T = nc.dram_tensor("wa_T", [D, D], BF16, kind="ExternalInput")       # W_ih_cl[:, :D]^T (rhs)
    wb_T = nc.dram_tensor("wb_T", [D, D], BF16, kind="ExternalInput")       # (W_ih_cl[:, D:])^T (lhsT)
    whh_cl_T = nc.dram_tensor("whh_cl_T", [D, D], BF16, kind="ExternalInput")
    bias_lc_in = nc.dram_tensor("bias_lc", [D, 1], F32, kind="ExternalInput")
    bias_cl_in = nc.dram_tensor("bias_cl", [D, 1], F32, kind="ExternalInput")
    ones_m_in = nc.dram_tensor("ones_m", [D, D], F32R, kind="ExternalInput")
    tv_in = nc.dram_tensor("tv", [D, 1], BF16, kind="ExternalInput")        # true_vec_w^T (lhsT)

    # ---- DRAM outputs ----
    clause_out = nc.dram_tensor("clause_out", [T, D, NCc], F32, kind="ExternalOutput")
    norm_c_out = nc.dram_tensor("norm_c_out", [T, NCc], F32, kind="ExternalOutput")
    truth_out = nc.dram_tensor("truth_out", [T, NLc], F32, kind="ExternalOutput")
    xl_fin_out = nc.dram_tensor("xl_fin_out", [D, NLc], F32, kind="ExternalOutput")

    NG = 8  # tiles per phase group

    with tile.TileContext(nc) as tc:
        with (
            tc.tile_pool(name="cst", bufs=1) as cst,
            tc.tile_pool(name="st", bufs=1) as st,
            tc.tile_pool(name="stg", bufs=NG + 1) as stg,
            tc.tile_pool(name="wk", bufs=3) as wk,
            tc.tile_pool(name="pmm", bufs=2, space="PSUM") as pmm,
            tc.tile_pool(name="psq", bufs=2, space="PSUM") as psq,
            tc.tile_pool(name="pzy", bufs=2, space="PSUM") as pzy,
            tc.tile_pool(name="pnr", bufs=2, space="PSUM") as pnr,
        ):
            # persistent tensors
            xlT = st.tile([D, NLc], BF16, tag="xlT")
            xcT = st.tile([D, NCc], BF16, tag="xcT")
            y_l = st.tile([D, NLc], BF16, tag="y_l")       # 32 normal-layout tiles
            z_c = st.tile([D, NCc], BF16, tag="z_c")       # 64 normal-layout tiles
            a_lc = cst.tile([D, GPC * 4 * CLS_PER], FP8, tag="a_lc")
            a_cl = cst.tile([D, GPC * 8 * LITS_PER], FP8, tag="a_cl")
            wyl = cst.tile([D, D], BF16, tag="wyl")
            whhlc = cst.tile([D, D], BF16, tag="whhlc")
            waT = cst.tile([D, D], BF16, tag="waT")
            wbT = cst.tile([D, D], BF16, tag="wbT")
            whhcl = cst.tile([D, D], BF16, tag="whhcl")
            b_lc = cst.tile([D, 1], F32, tag="b_lc")
            b_cl = cst.tile([D, 1], F32, tag="b_cl")
            ones_m = cst.tile([D, D], F32R, tag="ones_m")
            tv = cst.tile([D, 1], BF16, tag="tv")

            nc.gpsimd.dma_start(xlT[:], xl0T[:])
            nc.gpsimd.dma_start(xcT[:], xc0T[:])
            nc.gpsimd.dma_start(a_lc[:], adj_lc[:])
            nc.gpsimd.dma_start(a_cl[:], adj_cl[:])
            nc.gpsimd.dma_start(wyl[:], w_yl[:])
            nc.gpsimd.dma_start(whhlc[:], whh_lc_T[:])
            nc.gpsimd.dma_start(waT[:], wa_T[:])
            nc.gpsimd.dma_start(wbT[:], wb_T[:])
            nc.gpsimd.dma_start(whhcl[:], whh_cl_T[:])
            nc.gpsimd.dma_start(b_lc[:], bias_lc_in[:])
            nc.gpsimd.dma_start(b_cl[:], bias_cl_in[:])
            nc.gpsimd.dma_start(ones_m[:], ones_m_in[:])
            nc.gpsimd.dma_start(tv[:], tv_in[:])

            def clause_head(t, n):
                """matmuls + tanh + snapshot + sumsq + recip -> (stage, rcp)"""
                g, half = divmod(n, 2)
                ps = pmm.tile([D, 512], F32, tag="mm")
                for kt in range(4):
                    nc.tensor.matmul(
                        ps[:],
                        y_l[:, ts(g * 4 + kt, D)],
                        a_lc[:, g * 4 * CLS_PER + kt * CLS_PER + half * 512:][:, :512],
                        start=(kt == 0),
                        stop=False,
                    )
                nc.tensor.matmul(ps[:], whhlc[:], xcT[:, ts(n, 512)], start=False, stop=True)
                stage = stg.tile([D, 512], F32, tag="stage")
                nc.scalar.activation(stage[:], ps[:], AF.Tanh, bias=b_lc[:])
                nc.sync.dma_start(clause_out[t, :, ts(n, 512)], stage[:])
                sq = wk.tile([D, 512], F32R, tag="sq")
                nc.gpsimd.tensor_tensor(sq[:], stage[:], stage[:], op=mybir.AluOpType.mult)
                pq = psq.tile([D, 512], F32, tag="sq")
                nc.tensor.matmul(pq[:], ones_m[:], sq[:], start=True, stop=True)
                rcp = stg.tile([D, 512], F32, tag="rcp")
                nc.vector.reciprocal(rcp[:], pq[:])
                return stage, rcp

            def clause_tail(t, n, stage, rcp):
                """sqrt + state update + z_c"""
                inv = wk.tile([D, 512], F32, tag="inv")
                nc.scalar.activation(inv[:], rcp[:], AF.Sqrt)
                nc.sync.dma_start(norm_c_out[t:t + 1, ts(n, 512)], rcp[:1, :])
                nc.vector.tensor_mul(xcT[:, ts(n, 512)], stage[:], inv[:])
                xcb = wk.tile([D, 512], BF16, tag="xcb")
                nc.vector.tensor_copy(xcb[:], stage[:])
                pz = pzy.tile([D, 512], F32, tag="zy")
                for q in range(4):
                    nc.tensor.matmul(pz[:, ts(q, D)], xcb[:, ts(q, D)], waT[:],
                                     start=True, stop=True)
                nc.vector.tensor_copy(z_c[:, ts(n, 512)], pz[:])

            def lit_head(t, g):
                ps = pmm.tile([D, 512], F32, tag="mm")
                for kt in range(8):
                    nc.tensor.matmul(
                        ps[:],
                        z_c[:, ts(g * 8 + kt, D)],
                        a_cl[:, g * 8 * LITS_PER + kt * LITS_PER:][:, :512],
                        start=(kt == 0),
                        stop=False,
                    )
                o = g * LITS_PER
                nc.tensor.matmul(ps[:, 0:256], wbT[:], xlT[:, o + 256:o + 512],
                                 start=False, stop=False)
                nc.tensor.matmul(ps[:, 256:512], wbT[:], xlT[:, o:o + 256],
                                 start=False, stop=False)
                nc.tensor.matmul(ps[:], whhcl[:], xlT[:, ts(g, 512)], start=False, stop=True)
                stage = stg.tile([D, 512], F32, tag="stage")
                nc.scalar.activation(stage[:], ps[:], AF.Tanh, bias=b_cl[:])
                sq = wk.tile([D, 512], F32R, tag="sq")
                nc.gpsimd.tensor_tensor(sq[:], stage[:], stage[:], op=mybir.AluOpType.mult)
                pq = psq.tile([D, 512], F32, tag="sq")
                nc.tensor.matmul(pq[:], ones_m[:], sq[:], start=True, stop=True)
                rcp = stg.tile([D, 512], F32, tag="rcp")
                nc.vector.reciprocal(rcp[:], pq[:])
                return stage, rcp

            def lit_tail(t, g, stage, rcp):
                inv = wk.tile([D, 512], F32, tag="inv")
                nc.scalar.activation(inv[:], rcp[:], AF.Sqrt)
                nc.vector.tensor_mul(xlT[:, ts(g, 512)], stage[:], inv[:])
                if t == T - 1:
                    xlf = wk.tile([D, 512], F32, tag="xlf")
                    nc.vector.tensor_mul(xlf[:], stage[:], inv[:])
                    nc.sync.dma_start(xl_fin_out[:, ts(g, 512)], xlf[:])
                pst = pnr.tile([1, 512], F32, tag="nrm")
                nc.tensor.matmul(pst[:], tv[:], xlT[:, ts(g, 512)], start=True, stop=True)
                trs = wk.tile([1, 512], F32, tag="trs")
                nc.scalar.activation(trs[:], pst[:], AF.Copy)
                nc.sync.dma_start(truth_out[t:t + 1, ts(g, 512)], trs[:1, :])

            for t in range(T):
                # ---- y_l = x_l @ W_ih_lc^T (normal layout, 32 x [128,128]) ----
                for j4 in range(NLc // 512):
                    py = pzy.tile([D, 512], F32, tag="zy")
                    for q in range(4):
                        nc.tensor.matmul(py[:, ts(q, D)], xlT[:, ts(j4 * 4 + q, D)],
                                         wyl[:], start=True, stop=True)
                    nc.scalar.activation(y_l[:, ts(j4, 512)], py[:], AF.Copy)

                # ---- clause update, phase-grouped ----
                for grp in range(NCc // 512 // NG):
                    heads = [clause_head(t, grp * NG + i) for i in range(NG)]
                    for i, (stage, rcp) in enumerate(heads):
                        clause_tail(t, grp * NG + i, stage, rcp)

                # ---- literal update ----
                heads = [lit_head(t, g) for g in range(GPC)]
                for g, (stage, rcp) in enumerate(heads):
                    lit_tail(t, g, stage, rcp)

    nc.finalize()
    return nc


def _prep_inputs(edge_clause, edge_lit, x_l0, x_c0,
                 W_ih_lc, W_hh_lc, b_ih_lc, b_hh_lc,
                 W_ih_cl, W_hh_cl, b_ih_cl, b_hh_cl, true_vec_w):
    bf = ml_dtypes.bfloat16
    f8 = ml_dtypes.float8_e4m3fn
    xl_n = x_l0 / np.linalg.norm(x_l0, axis=1, keepdims=True)
    xc_n = x_c0 / np.linalg.norm(x_c0, axis=1, keepdims=True)

    lit_local = (edge_lit % LITS_PER).astype(np.int64)
    A = np.zeros((NC, LITS_PER), np.float32)
    np.add.at(A, (edge_clause.astype(np.int64), lit_local), 1.0)

    in_maps = []
    shared = {
        "w_yl": W_ih_lc.T.copy().astype(bf),
        "whh_lc_T": W_hh_lc.T.copy().astype(bf),
        "wa_T": W_ih_cl[:, :D].T.copy().astype(bf),
        "wb_T": W_ih_cl[:, D:].T.copy().astype(bf),
        "whh_cl_T": W_hh_cl.T.copy().astype(bf),
        "bias_lc": (b_ih_lc + b_hh_lc).reshape(D, 1).astype(np.float32),
        "bias_cl": (b_ih_cl + b_hh_cl).reshape(D, 1).astype(np.float32),
        "ones_m": np.ones((D, D), np.float32),
        "tv": true_vec_w.reshape(1, D).T.copy().astype(bf),
    }
    for m in range(M):
        lit0, cl0 = m * NLc, m * NCc
        xlT = np.ascontiguousarray(xl_n[lit0:lit0 + NLc].T).astype(bf)
        xcT = np.ascontiguousarray(xc_n[cl0:cl0 + NCc].T).astype(bf)
        # adj_lc: per graph g, kt in 0..3: A_g.T[kt*128:(kt+1)*128, :]  [128, 1024]
        a_lc = np.empty((D, GPC * 4 * CLS_PER), np.float32)
        a_cl = np.empty((D, GPC * 8 * LITS_PER), np.float32)
        for g in range(GPC):
            Ag = A[cl0 + g * CLS_PER: cl0 + (g + 1) * CLS_PER]       # [1024, 512]
            AgT = Ag.T                                                # [512, 1024]
            for kt in range(4):
                a_lc[:, (g * 4 + kt) * CLS_PER:(g * 4 + kt + 1) * CLS_PER] = \
                    AgT[kt * D:(kt + 1) * D]
            for kt in range(8):
                a_cl[:, (g * 8 + kt) * LITS_PER:(g * 8 + kt + 1) * LITS_PER] = \
                    Ag[kt * D:(kt + 1) * D]
        in_maps.append({
            "xl0T": xlT, "xc0T": xcT,
            "adj_lc": a_lc.astype(f8), "adj_cl": a_cl.astype(f8),
            **shared,
        })
    return in_maps, xl_n, xc_n


def kernel(edge_clause, edge_lit, x_l0, x_c0,
           W_ih_lc, W_hh_lc, b_ih_lc, b_hh_lc,
           W_ih_cl, W_hh_cl, b_ih_cl, b_hh_cl,
           L_vote_w, L_vote_b, true_vec_w, num_iters, **kw):
    edge_clause = np.asarray(edge_clause)
    edge_lit = np.asarray(edge_lit)
    x_l0 = np.asarray(x_l0, np.float32)
    x_c0 = np.asarray(x_c0, np.float32)
    W_ih_lc = np.asarray(W_ih_lc, np.float32)
    W_hh_lc = np.asarray(W_hh_lc, np.float32)
    b_ih_lc = np.asarray(b_ih_lc, np.float32)
    b_hh_lc = np.asarray(b_hh_lc, np.float32)
    W_ih_cl = np.asarray(W_ih_cl, np.float32)
    W_hh_cl = np.asarray(W_hh_cl, np.float32)
    b_ih_cl = np.asarray(b_ih_cl, np.float32)
    b_hh_cl = np.asarray(b_hh_cl, np.float32)
    L_vote_w = np.asarray(L_vote_w, np.float32)
    L_vote_b = np.asarray(L_vote_b, np.float32)
    true_vec_w = np.asarray(true_vec_w, np.float32)
    assert int(np.asarray(num_iters)) == T

    in_maps, xl_n0, xc_n0 = _prep_inputs(
        edge_clause, edge_lit, x_l0, x_c0, W_ih_lc, W_hh_lc, b_ih_lc, b_hh_lc,
        W_ih_cl, W_hh_cl, b_ih_cl, b_hh_cl, true_vec_w)

    if "nc" not in _CACHE:
        _CACHE["nc"] = _build_kernel()
    res = run_bass_kernel_spmd(_CACHE["nc"], in_maps, core_ids=list(range(M)),
                               trace=PROFILE)
    global LAST_RESULTS
    LAST_RESULTS = res

    # ---- host-side assembly ----
    truth_all = np.empty((T + 1, NL, 1), np.float32)
    clause_all = np.empty((T + 1, NC, D), np.float32)
    x_l = np.empty((NL, D), np.float32)
    truth_all[0] = (xl_n0 @ true_vec_w.T).reshape(NL, 1)
    clause_all[0] = xc_n0
    for m in range(M):
        r = res.results[m]
        lit0, cl0 = m * NLc, m * NCc
        truth_all[1:, lit0:lit0 + NLc, 0] = r["truth_out"]
        # clause snapshots: unnormalized [T, 128, NCc] * rsqrt(recip sumsq)
        inv = np.sqrt(r["norm_c_out"])          # rcp = 1/sum(x^2); inv = 1/norm
        snap = r["clause_out"] * inv[:, None, :]
        clause_all[1:, cl0:cl0 + NCc, :] = snap.transpose(0, 2, 1)
        x_l[lit0:lit0 + NLc] = r["xl_fin_out"].T

    x_l_vote = x_l @ L_vote_w.T + L_vote_b
    vote_mean_pool = x_l_vote.reshape(B, LITS_PER, 1).mean(axis=1)
    return (x_l_vote, x_l, vote_mean_pool, truth_all[-1], truth_all,
            clause_all, truth_all[0])


# revision 12
# speedup vs baseline: 1.9068x; 1.5959x over previous
"""NeuroSAT message-passing RNN on 8 Trainium2 NeuronCores.

Per core (8 graphs: 4096 literals, 8192 clauses), all state resident in SBUF
in TRANSPOSED layout [D=128 partitions, nodes], states in bf16:
- Sparse clause<->literal segment-sums run on the TensorEngine as block-dense
  count matmuls against per-graph adjacency blocks in fp8 (counts exact),
  fused with the RNN input projections:
    LC:  pre_c = (W_ih_lc @ x_l^T) @ A^T + W_hh_lc @ x_c^T
    CL:  pre_l = (W_A @ x_cnew^T) @ A + W_B @ flip(x_l)^T + W_hh_cl @ x_l^T
  where y_l = x_l @ W_ih_lc^T and z_c = x_cnew @ W_A^T are materialized in
  normal layout via N=128 matmuls (lhsT = transposed-state slices).
- tanh+bias on ScalarE (PSUM -> bf16); per-column sum-of-squares via an
  all-ones [128,128] f32r matmul producing the partition-broadcast result;
  1/sumsq via DVE reciprocal_approx_fast; 1/norm via one batched ACT Sqrt per
  phase (avoids ACT table thrash); squares on GpSimd.
- Clause snapshots leave unnormalized in bf16 + per-column 1/sumsq rows; the
  host normalizes. Literal states are exported per iteration (bf16) and truth
  projections computed on host; final x_l is normalized on-chip in fp32.
"""

import numpy as np
import ml_dtypes

import concourse.bass as bass
import concourse.tile as tile
from concourse import bacc, mybir
from concourse.bass import ts
from concourse.bass_utils import run_bass_kernel_spmd

F32 = mybir.dt.float32
F32R = mybir.dt.float32r
BF16 = mybir.dt.bfloat16
FP8 = mybir.dt.float8e4
AF = mybir.ActivationFunctionType
MUL = mybir.AluOpType.mult

B = 64
LITS_PER = 512
CLS_PER = 1024
NL = B * LITS_PER
NC = B * CLS_PER
D = 128
T = 8
M = 8                    # cores
GPC = B // M             # graphs per core
NLc = GPC * LITS_PER     # 4096
NCc = GPC * CLS_PER      # 8192
W = 1024                 # working tile width

_CACHE = {}
PROFILE = False
LAST_RESULTS = None


def _build_kernel():
    nc = bacc.Bacc(None, target_bir_lowering=False)

    # ---- DRAM inputs ----
    xl0T = nc.dram_tensor("xl0T", [D, NLc], BF16, kind="ExternalInput")
    xc0T = nc.dram_tensor("xc0T", [D, NCc], BF16, kind="ExternalInput")
    adj_lc = nc.dram_tensor("adj_lc", [D, GPC * 4 * CLS_PER], FP8, kind="ExternalInput")
    adj_cl = nc.dram_tensor("adj_cl", [D, GPC * 8 * LITS_PER], FP8, kind="ExternalInput")
    w_yl = nc.dram_tensor("w_yl", [D, D], BF16, kind="ExternalInput")       # W_ih_lc^T (rhs)
    whh_lc_T = nc.dram_tensor("whh_lc_T", [D, D], BF16, kind="ExternalInput")
    wa_T = nc.dram_tensor("wa_T", [D, D], BF16, kind="ExternalInput")       # W_ih_cl[:, :D]^T (rhs)
    wb_T = nc.dram_tensor("wb_T", [D, D], BF16, kind="ExternalInput")       # (W_ih_cl[:, D:])^T (lhsT)
    whh_cl_T = nc.dram_tensor("whh_cl_T", [D, D], BF16, kind="ExternalInput")
    bias_lc_in = nc.dram_tensor("bias_lc", [D, 1], F32, kind="ExternalInput")
    bias_cl_in = nc.dram_tensor("bias_cl", [D, 1], F32, kind="ExternalInput")
    ones_m_in = nc.dram_tensor("ones_m", [D, D], F32R, kind="ExternalInput")

    # ---- DRAM outputs ----
    clause_out = nc.dram_tensor("clause_out", [T, D, NCc], BF16, kind="ExternalOutput")
    rcp_c_out = nc.dram_tensor("rcp_c_out", [T, NCc], F32, kind="ExternalOutput")
    xl_state_out = nc.dram_tensor("xl_state_out", [T, D, NLc], BF16, kind="ExternalOutput")
    xl_fin_out = nc.dram_tensor("xl_fin_out", [D, NLc], F32, kind="ExternalOutput")

    with tile.TileContext(nc) as tc:
        with (
            tc.tile_pool(name="cst", bufs=1) as cst,
            tc.tile_pool(name="st", bufs=1) as st,
            tc.tile_pool(name="stg", bufs=6) as stg,
            tc.tile_pool(name="wk", bufs=3) as wk,
            tc.tile_pool(name="pmm", bufs=2, space="PSUM") as pmm,
            tc.tile_pool(name="psq", bufs=1, space="PSUM") as psq,
            tc.tile_pool(name="pzy", bufs=1, space="PSUM") as pzy,
        ):
            # persistent tensors
            xlT = st.tile([D, NLc], BF16, tag="xlT")
            xcT = st.tile([D, NCc], BF16, tag="xcT")
            y_l = st.tile([D, NLc], BF16, tag="y_l")       # 32 normal-layout tiles
            z_c = st.tile([D, NCc], BF16, tag="z_c")       # 64 normal-layout tiles
            rcp = st.tile([D, NLc], F32, tag="rcp")        # 1/sumsq, bcast rows
            inv = st.tile([D, NLc], BF16, tag="inv")       # 1/norm, bcast rows
            a_lc = cst.tile([D, GPC * 4 * CLS_PER], FP8, tag="a_lc")
            a_cl = cst.tile([D, GPC * 8 * LITS_PER], FP8, tag="a_cl")
            wyl = cst.tile([D, D], BF16, tag="wyl")
            whhlc = cst.tile([D, D], BF16, tag="whhlc")
            waT = cst.tile([D, D], BF16, tag="waT")
            wbT = cst.tile([D, D], BF16, tag="wbT")
            whhcl = cst.tile([D, D], BF16, tag="whhcl")
            b_lc = cst.tile([D, 1], F32, tag="b_lc")
            b_cl = cst.tile([D, 1], F32, tag="b_cl")
            ones_m = cst.tile([D, D], F32R, tag="ones_m")

            nc.gpsimd.dma_start(xlT[:], xl0T[:])
            nc.gpsimd.dma_start(xcT[:], xc0T[:])
            nc.gpsimd.dma_start(a_lc[:], adj_lc[:])
            nc.gpsimd.dma_start(a_cl[:], adj_cl[:])
            nc.gpsimd.dma_start(wyl[:], w_yl[:])
            nc.gpsimd.dma_start(whhlc[:], whh_lc_T[:])
            nc.gpsimd.dma_start(waT[:], wa_T[:])
            nc.gpsimd.dma_start(wbT[:], wb_T[:])
            nc.gpsimd.dma_start(whhcl[:], whh_cl_T[:])
            nc.gpsimd.dma_start(b_lc[:], bias_lc_in[:])
            nc.gpsimd.dma_start(b_cl[:], bias_cl_in[:])
            nc.gpsimd.dma_start(ones_m[:], ones_m_in[:])

            def norm_head(stage, off):
                """sq -> sumsq (bcast matmul) -> reciprocal into rcp[:, off:off+W]."""
                sq = wk.tile([D, W], F32R, tag="sq")
                nc.gpsimd.tensor_tensor(sq[:], stage[:], stage[:], op=MUL)
                pq = psq.tile([D, W], F32, tag="sq")
                nc.tensor.matmul(pq[:, 0:512], ones_m[:], sq[:, 0:512], start=True, stop=True)
                nc.tensor.matmul(pq[:, 512:W], ones_m[:], sq[:, 512:W], start=True, stop=True)
                nc.vector.reciprocal_approx_fast(rcp[:, off:off + W], pq[:])

            def clause_head(t, n):
                """n-th [128, 1024] clause tile: matmuls + tanh + snapshot + norm."""
                g = n  # one 1024-tile per... (2 per graph at 512) -> g = n // 1? W=1024: graph = n // 1
                ps = pmm.tile([D, W], F32, tag="mm")
                for h in range(2):
                    gg, half = divmod(2 * n + h, 2)
                    sl = slice(h * 512, h * 512 + 512)
                    for kt in range(4):
                        nc.tensor.matmul(
                            ps[:, sl],
                            y_l[:, ts(gg * 4 + kt, D)],
                            a_lc[:, gg * 4 * CLS_PER + kt * CLS_PER + half * 512:][:, :512],
                            start=(kt == 0),
                            stop=False,
                        )
                    nc.tensor.matmul(ps[:, sl], whhlc[:], xcT[:, 2 * n * 512 + h * 512:][:, :512],
                                     start=False, stop=True)
                stage = stg.tile([D, W], BF16, tag="stage")
                nc.scalar.activation(stage[:], ps[:], AF.Tanh, bias=b_lc[:])
                nc.sync.dma_start(clause_out[t, :, ts(n, W)], stage[:])
                norm_head(stage, (n % 4) * W)
                return stage

            def clause_tail(t, n, stage):
                nc.vector.tensor_mul(xcT[:, ts(n, W)], stage[:], inv[:, ts(n % 4, W)])
                pz = pzy.tile([D, W], F32, tag="zy")
                for q in range(8):
                    nc.tensor.matmul(pz[:, ts(q, D)], stage[:, ts(q, D)], waT[:],
                                     start=True, stop=True)
                nc.vector.tensor_copy(z_c[:, ts(n, W)], pz[:])

            def lit_head(t, n):
                """n-th [128, 1024] literal tile covers graphs 2n, 2n+1."""
                ps = pmm.tile([D, W], F32, tag="mm")
                for h in range(2):
                    g = 2 * n + h
                    sl = slice(h * 512, h * 512 + 512)
                    for kt in range(8):
                        nc.tensor.matmul(
                            ps[:, sl],
                            z_c[:, ts(g * 8 + kt, D)],
                            a_cl[:, g * 8 * LITS_PER + kt * LITS_PER:][:, :512],
                            start=(kt == 0),
                            stop=False,
                        )
                    o = g * LITS_PER
                    nc.tensor.matmul(ps[:, h * 512:h * 512 + 256], wbT[:],
                                     xlT[:, o + 256:o + 512], start=False, stop=False)
                    nc.tensor.matmul(ps[:, h * 512 + 256:h * 512 + 512], wbT[:],
                                     xlT[:, o:o + 256], start=False, stop=False)
                    nc.tensor.matmul(ps[:, sl], whhcl[:], xlT[:, o:o + 512],
                                     start=False, stop=True)
                stage = stg.tile([D, W], BF16, tag="stage")
                nc.scalar.activation(stage[:], ps[:], AF.Tanh, bias=b_cl[:])
                norm_head(stage, n * W)
                return stage

            def lit_tail(t, n, stage):
                nc.vector.tensor_mul(xlT[:, ts(n, W)], stage[:], inv[:, ts(n, W)])
                if t == T - 1:
                    xlf = wk.tile([D, W], F32, tag="xlf")
                    nc.vector.tensor_mul(xlf[:], stage[:], inv[:, ts(n, W)])
                    nc.sync.dma_start(xl_fin_out[:, ts(n, W)], xlf[:])

            for t in range(T):
                # ---- y_l = x_l @ W_ih_lc^T (normal layout, 32 x [128,128]) ----
                for j8 in range(NLc // W):
                    py = pzy.tile([D, W], F32, tag="zy")
                    for q in range(8):
                        nc.tensor.matmul(py[:, ts(q, D)], xlT[:, ts(j8 * 8 + q, D)],
                                         wyl[:], start=True, stop=True)
                    nc.scalar.activation(y_l[:, ts(j8, W)], py[:], AF.Copy)

                # ---- clause update: 2 half-batches of 4 tiles ----
                for half in range(2):
                    ns = range(half * 4, half * 4 + 4)
                    stages = [clause_head(t, n) for n in ns]
                    nc.scalar.activation(inv[:], rcp[:], AF.Sqrt)
                    nc.sync.dma_start(rcp_c_out[t:t + 1, ts(half, NLc)], rcp[:1, :])
                    for n, stage in zip(ns, stages):
                        clause_tail(t, n, stage)

                # ---- literal update: 4 heads, one batched sqrt, 4 tails ----
                stages = [lit_head(t, n) for n in range(NLc // W)]
                nc.scalar.activation(inv[:, :NLc], rcp[:, :NLc], AF.Sqrt)
                for n, stage in enumerate(stages):
                    lit_tail(t, n, stage)
                nc.sync.dma_start(xl_state_out[t, :, :], xlT[:])

    nc.finalize()
    return nc


def _prep_inputs(edge_clause, edge_lit, x_l0, x_c0,
                 W_ih_lc, W_hh_lc, b_ih_lc, b_hh_lc,
                 W_ih_cl, W_hh_cl, b_ih_cl, b_hh_cl, true_vec_w):
    bf = ml_dtypes.bfloat16
    f8 = ml_dtypes.float8_e4m3fn
    xl_n = x_l0 / np.linalg.norm(x_l0, axis=1, keepdims=True)
    xc_n = x_c0 / np.linalg.norm(x_c0, axis=1, keepdims=True)

    lit_local = (edge_lit % LITS_PER).astype(np.int64)
    A = np.zeros((NC, LITS_PER), np.float32)
    np.add.at(A, (edge_clause.astype(np.int64), lit_local), 1.0)

    in_maps = []
    shared = {
        "w_yl": W_ih_lc.T.copy().astype(bf),
        "whh_lc_T": W_hh_lc.T.copy().astype(bf),
        "wa_T": W_ih_cl[:, :D].T.copy().astype(bf),
        "wb_T": W_ih_cl[:, D:].T.copy().astype(bf),
        "whh_cl_T": W_hh_cl.T.copy().astype(bf),
        "bias_lc": (b_ih_lc + b_hh_lc).reshape(D, 1).astype(np.float32),
        "bias_cl": (b_ih_cl + b_hh_cl).reshape(D, 1).astype(np.float32),
        "ones_m": np.ones((D, D), np.float32),
    }
    for m in range(M):
        lit0, cl0 = m * NLc, m * NCc
        xlT = np.ascontiguousarray(xl_n[lit0:lit0 + NLc].T).astype(bf)
        xcT = np.ascontiguousarray(xc_n[cl0:cl0 + NCc].T).astype(bf)
        a_lc = np.empty((D, GPC * 4 * CLS_PER), np.float32)
        a_cl = np.empty((D, GPC * 8 * LITS_PER), np.float32)
        for g in range(GPC):
            Ag = A[cl0 + g * CLS_PER: cl0 + (g + 1) * CLS_PER]       # [1024, 512]
            AgT = Ag.T                                                # [512, 1024]
            for kt in range(4):
                a_lc[:, (g * 4 + kt) * CLS_PER:(g * 4 + kt + 1) * CLS_PER] = \
                    AgT[kt * D:(kt + 1) * D]
            for kt in range(8):
                a_cl[:, (g * 8 + kt) * LITS_PER:(g * 8 + kt + 1) * LITS_PER] = \
                    Ag[kt * D:(kt + 1) * D]
        in_maps.append({
            "xl0T": xlT, "xc0T": xcT,
            "adj_lc": a_lc.astype(f8), "adj_cl": a_cl.astype(f8),
            **shared,
        })
    return in_maps, xl_n, xc_n


def kernel(edge_clause, edge_lit, x_l0, x_c0,
           W_ih_lc, W_hh_lc, b_ih_lc, b_hh_lc,
           W_ih_cl, W_hh_cl, b_ih_cl, b_hh_cl,
           L_vote_w, L_vote_b, true_vec_w, num_iters, **kw):
    edge_clause = np.asarray(edge_clause)
    edge_lit = np.asarray(edge_lit)
    x_l0 = np.asarray(x_l0, np.float32)
    x_c0 = np.asarray(x_c0, np.float32)
    W_ih_lc = np.asarray(W_ih_lc, np.float32)
    W_hh_lc = np.asarray(W_hh_lc, np.float32)
    b_ih_lc = np.asarray(b_ih_lc, np.float32)
    b_hh_lc = np.asarray(b_hh_lc, np.float32)
    W_ih_cl = np.asarray(W_ih_cl, np.float32)
    W_hh_cl = np.asarray(W_hh_cl, np.float32)
    b_ih_cl = np.asarray(b_ih_cl, np.float32)
    b_hh_cl = np.asarray(b_hh_cl, np.float32)
    L_vote_w = np.asarray(L_vote_w, np.float32)
    L_vote_b = np.asarray(L_vote_b, np.float32)
    true_vec_w = np.asarray(true_vec_w, np.float32)
    assert int(np.asarray(num_iters)) == T

    in_maps, xl_n0, xc_n0 = _prep_inputs(
        edge_clause, edge_lit, x_l0, x_c0, W_ih_lc, W_hh_lc, b_ih_lc, b_hh_lc,
        W_ih_cl, W_hh_cl, b_ih_cl, b_hh_cl, true_vec_w)

    if "nc" not in _CACHE:
        _CACHE["nc"] = _build_kernel()
    res = run_bass_kernel_spmd(_CACHE["nc"], in_maps, core_ids=list(range(M)),
                               trace=PROFILE)
    global LAST_RESULTS
    LAST_RESULTS = res

    # ---- host-side assembly ----
    truth_all = np.empty((T + 1, NL, 1), np.float32)
    clause_all = np.empty((T + 1, NC, D), np.float32)
    x_l = np.empty((NL, D), np.float32)
    truth_all[0] = (xl_n0 @ true_vec_w.T).reshape(NL, 1)
    clause_all[0] = xc_n0
    tvec = true_vec_w.reshape(D).astype(np.float32)
    for m in range(M):
        r = res.results[m]
        lit0, cl0 = m * NLc, m * NCc
        # truth from exported bf16 literal states
        xs = r["xl_state_out"].astype(np.float32)          # [T, D, NLc]
        truth_all[1:, lit0:lit0 + NLc, 0] = np.einsum("tdn,d->tn", xs, tvec)
        inv = np.sqrt(r["rcp_c_out"])                       # 1/norm per clause col
        snap = r["clause_out"].astype(np.float32) * inv[:, None, :]
        clause_all[1:, cl0:cl0 + NCc, :] = snap.transpose(0, 2, 1)
        x_l[lit0:lit0 + NLc] = r["xl_fin_out"].T

    x_l_vote = x_l @ L_vote_w.T + L_vote_b
    vote_mean_pool = x_l_vote.reshape(B, LITS_PER, 1).mean(axis=1)
    return (x_l_vote, x_l, vote_mean_pool, truth_all[-1], truth_all,
            clause_all, truth_all[0])


# revision 13
# speedup vs baseline: 1.9742x; 1.0353x over previous
"""NeuroSAT message-passing RNN on 8 Trainium2 NeuronCores.

Per core (8 graphs: 4096 literals, 8192 clauses), all state resident in SBUF
in TRANSPOSED layout [D=128 partitions, nodes], states in bf16:
- Sparse clause<->literal segment-sums run on the TensorEngine as block-dense
  count matmuls against per-graph adjacency blocks in fp8 (counts exact),
  fused with the RNN input projections:
    LC:  pre_c = (W_ih_lc @ x_l^T) @ A^T + W_hh_lc @ x_c^T
    CL:  pre_l = (W_A @ x_cnew^T) @ A + W_B @ flip(x_l)^T + W_hh_cl @ x_l^T
  where y_l = x_l @ W_ih_lc^T and z_c = x_cnew @ W_A^T are materialized in
  normal layout via N=128 matmuls (lhsT = transposed-state slices).
- tanh+bias on ScalarE (PSUM -> bf16); per-column sum-of-squares via an
  all-ones [128,128] f32r matmul producing the partition-broadcast result;
  1/sumsq via DVE reciprocal_approx_fast; 1/norm via one batched ACT Sqrt per
  phase (avoids ACT table thrash); squares on GpSimd.
- Clause snapshots leave unnormalized in bf16 + per-column 1/sumsq rows; the
  host normalizes. Literal states are exported per iteration (bf16) and truth
  projections computed on host; final x_l is normalized on-chip in fp32.
"""

import numpy as np
import ml_dtypes

import concourse.bass as bass
import concourse.tile as tile
from concourse import bacc, mybir
from concourse.bass import ts
from concourse.bass_utils import run_bass_kernel_spmd

F32 = mybir.dt.float32
F32R = mybir.dt.float32r
BF16 = mybir.dt.bfloat16
FP8 = mybir.dt.float8e4
AF = mybir.ActivationFunctionType
MUL = mybir.AluOpType.mult

B = 64
LITS_PER = 512
CLS_PER = 1024
NL = B * LITS_PER
NC = B * CLS_PER
D = 128
T = 8
M = 8                    # cores
GPC = B // M             # graphs per core
NLc = GPC * LITS_PER     # 4096
NCc = GPC * CLS_PER      # 8192
W = 1024                 # working tile width

_CACHE = {}
PROFILE = False
LAST_RESULTS = None


def _build_kernel():
    nc = bacc.Bacc(None, target_bir_lowering=False)

    # ---- DRAM inputs ----
    xl0T = nc.dram_tensor("xl0T", [D, NLc], BF16, kind="ExternalInput")
    xc0T = nc.dram_tensor("xc0T", [D, NCc], BF16, kind="ExternalInput")
    adj_lc = nc.dram_tensor("adj_lc", [D, GPC * 4 * CLS_PER], FP8, kind="ExternalInput")
    adj_cl = nc.dram_tensor("adj_cl", [D, GPC * 8 * LITS_PER], FP8, kind="ExternalInput")
    w_yl = nc.dram_tensor("w_yl", [D, D], BF16, kind="ExternalInput")       # W_ih_lc^T (rhs)
    whh_lc_T = nc.dram_tensor("whh_lc_T", [D, D], BF16, kind="ExternalInput")
    wa_T = nc.dram_tensor("wa_T", [D, D], BF16, kind="ExternalInput")       # W_ih_cl[:, :D]^T (rhs)
    wb_T = nc.dram_tensor("wb_T", [D, D], BF16, kind="ExternalInput")       # (W_ih_cl[:, D:])^T (lhsT)
    whh_cl_T = nc.dram_tensor("whh_cl_T", [D, D], BF16, kind="ExternalInput")
    bias_lc_in = nc.dram_tensor("bias_lc", [D, 1], F32, kind="ExternalInput")
    bias_cl_in = nc.dram_tensor("bias_cl", [D, 1], F32, kind="ExternalInput")
    ones_m_in = nc.dram_tensor("ones_m", [D, D], F32R, kind="ExternalInput")

    # ---- DRAM outputs ----
    clause_out = nc.dram_tensor("clause_out", [T, D, NCc], BF16, kind="ExternalOutput")
    rcp_c_out = nc.dram_tensor("rcp_c_out", [T, NCc], F32, kind="ExternalOutput")
    xl_state_out = nc.dram_tensor("xl_state_out", [T, D, NLc], BF16, kind="ExternalOutput")
    xl_fin_out = nc.dram_tensor("xl_fin_out", [D, NLc], F32, kind="ExternalOutput")

    with tile.TileContext(nc) as tc:
        with (
            tc.tile_pool(name="cst", bufs=1) as cst,
            tc.tile_pool(name="st", bufs=1) as st,
            tc.tile_pool(name="stg", bufs=6) as stg,
            tc.tile_pool(name="wk", bufs=3) as wk,
            tc.tile_pool(name="pmm", bufs=2, space="PSUM") as pmm,
            tc.tile_pool(name="psq", bufs=2, space="PSUM") as psq,
            tc.tile_pool(name="pzy", bufs=1, space="PSUM") as pzy,
        ):
            # persistent tensors
            xlT = st.tile([D, NLc], BF16, tag="xlT")
            xcT = st.tile([D, NCc], BF16, tag="xcT")
            y_l = st.tile([D, NLc], BF16, tag="y_l")       # 32 normal-layout tiles
            z_c = st.tile([D, NCc], BF16, tag="z_c")       # 64 normal-layout tiles
            rcp = st.tile([D, NLc], F32, tag="rcp")        # 1/sumsq, bcast rows
            inv = st.tile([D, NLc], BF16, tag="inv")       # 1/norm, bcast rows
            a_lc = cst.tile([D, GPC * 4 * CLS_PER], FP8, tag="a_lc")
            a_cl = cst.tile([D, GPC * 8 * LITS_PER], FP8, tag="a_cl")
            wyl = cst.tile([D, D], BF16, tag="wyl")
            whhlc = cst.tile([D, D], BF16, tag="whhlc")
            waT = cst.tile([D, D], BF16, tag="waT")
            wbT = cst.tile([D, D], BF16, tag="wbT")
            whhcl = cst.tile([D, D], BF16, tag="whhcl")
            b_lc = cst.tile([D, 1], F32, tag="b_lc")
            b_cl = cst.tile([D, 1], F32, tag="b_cl")
            ones_m = cst.tile([D, D], F32R, tag="ones_m")

            nc.gpsimd.dma_start(xlT[:], xl0T[:])
            nc.gpsimd.dma_start(xcT[:], xc0T[:])
            nc.gpsimd.dma_start(a_lc[:], adj_lc[:])
            nc.gpsimd.dma_start(a_cl[:], adj_cl[:])
            nc.gpsimd.dma_start(wyl[:], w_yl[:])
            nc.gpsimd.dma_start(whhlc[:], whh_lc_T[:])
            nc.gpsimd.dma_start(waT[:], wa_T[:])
            nc.gpsimd.dma_start(wbT[:], wb_T[:])
            nc.gpsimd.dma_start(whhcl[:], whh_cl_T[:])
            nc.gpsimd.dma_start(b_lc[:], bias_lc_in[:])
            nc.gpsimd.dma_start(b_cl[:], bias_cl_in[:])
            nc.gpsimd.dma_start(ones_m[:], ones_m_in[:])

            def norm_head(stage, off):
                """sq -> sumsq (bcast matmul) -> reciprocal into rcp[:, off:off+W]."""
                sq = wk.tile([D, W], F32R, tag="sq")
                nc.gpsimd.tensor_tensor(sq[:], stage[:], stage[:], op=MUL)
                for h in range(2):
                    pq = psq.tile([D, 512], F32, tag="sq")
                    nc.tensor.matmul(pq[:], ones_m[:], sq[:, ts(h, 512)], start=True, stop=True)
                    nc.vector.reciprocal_approx_fast(rcp[:, off + h * 512:off + h * 512 + 512], pq[:])

            def clause_head(t, n):
                """n-th [128, 1024] clause tile: matmuls + tanh + snapshot + norm."""
                g = n  # one 1024-tile per... (2 per graph at 512) -> g = n // 1? W=1024: graph = n // 1
                ps = pmm.tile([D, W], F32, tag="mm")
                for h in range(2):
                    gg, half = divmod(2 * n + h, 2)
                    sl = slice(h * 512, h * 512 + 512)
                    for kt in range(4):
                        nc.tensor.matmul(
                            ps[:, sl],
                            y_l[:, ts(gg * 4 + kt, D)],
                            a_lc[:, gg * 4 * CLS_PER + kt * CLS_PER + half * 512:][:, :512],
                            start=(kt == 0),
                            stop=False,
                        )
                    nc.tensor.matmul(ps[:, sl], whhlc[:], xcT[:, 2 * n * 512 + h * 512:][:, :512],
                                     start=False, stop=True)
                stage = stg.tile([D, W], BF16, tag="stage")
                nc.scalar.activation(stage[:], ps[:], AF.Tanh, bias=b_lc[:])
                nc.sync.dma_start(clause_out[t, :, ts(n, W)], stage[:])
                norm_head(stage, (n % 4) * W)
                return stage

            def clause_tail(t, n, stage):
                pz = pzy.tile([D, W], F32, tag="zy")
                for q in range(8):
                    nc.tensor.matmul(pz[:, ts(q, D)], stage[:, ts(q, D)], waT[:],
                                     start=True, stop=True)
                nc.vector.tensor_copy(z_c[:, ts(n, W)], pz[:])
                nc.vector.tensor_mul(xcT[:, ts(n, W)], stage[:], inv[:, ts(n % 4, W)])

            def lit_head(t, n):
                """n-th [128, 1024] literal tile covers graphs 2n, 2n+1."""
                ps = pmm.tile([D, W], F32, tag="mm")
                for h in range(2):
                    g = 2 * n + h
                    sl = slice(h * 512, h * 512 + 512)
                    for kt in range(8):
                        nc.tensor.matmul(
                            ps[:, sl],
                            z_c[:, ts(g * 8 + kt, D)],
                            a_cl[:, g * 8 * LITS_PER + kt * LITS_PER:][:, :512],
                            start=(kt == 0),
                            stop=False,
                        )
                    o = g * LITS_PER
                    nc.tensor.matmul(ps[:, h * 512:h * 512 + 256], wbT[:],
                                     xlT[:, o + 256:o + 512], start=False, stop=False)
                    nc.tensor.matmul(ps[:, h * 512 + 256:h * 512 + 512], wbT[:],
                                     xlT[:, o:o + 256], start=False, stop=False)
                    nc.tensor.matmul(ps[:, sl], whhcl[:], xlT[:, o:o + 512],
                                     start=False, stop=True)
                stage = stg.tile([D, W], BF16, tag="stage")
                nc.scalar.activation(stage[:], ps[:], AF.Tanh, bias=b_cl[:])
                norm_head(stage, n * W)
                return stage

            def lit_tail(t, n, stage):
                nc.vector.tensor_mul(xlT[:, ts(n, W)], stage[:], inv[:, ts(n, W)])
                if t == T - 1:
                    xlf = wk.tile([D, W], F32, tag="xlf")
                    nc.vector.tensor_mul(xlf[:], stage[:], inv[:, ts(n, W)])
                    nc.sync.dma_start(xl_fin_out[:, ts(n, W)], xlf[:])
                else:
                    py = pzy.tile([D, W], F32, tag="zy")
                    for q in range(8):
                        nc.tensor.matmul(py[:, ts(q, D)], xlT[:, ts(n * 8 + q, D)],
                                         wyl[:], start=True, stop=True)
                    nc.scalar.activation(y_l[:, ts(n, W)], py[:], AF.Copy)

            for t in range(T):
                if t == 0:
                    # ---- y_l = x_l @ W_ih_lc^T (normal layout) ----
                    for j8 in range(NLc // W):
                        py = pzy.tile([D, W], F32, tag="zy")
                        for q in range(8):
                            nc.tensor.matmul(py[:, ts(q, D)], xlT[:, ts(j8 * 8 + q, D)],
                                             wyl[:], start=True, stop=True)
                        nc.scalar.activation(y_l[:, ts(j8, W)], py[:], AF.Copy)

                # ---- clause update: 2 half-batches of 4 tiles ----
                for half in range(2):
                    ns = range(half * 4, half * 4 + 4)
                    stages = [clause_head(t, n) for n in ns]
                    nc.scalar.activation(inv[:], rcp[:], AF.Sqrt)
                    nc.sync.dma_start(rcp_c_out[t:t + 1, ts(half, NLc)], rcp[:1, :])
                    for n, stage in zip(ns, stages):
                        clause_tail(t, n, stage)

                # ---- literal update: 4 heads, one batched sqrt, 4 tails ----
                stages = [lit_head(t, n) for n in range(NLc // W)]
                nc.scalar.activation(inv[:, :NLc], rcp[:, :NLc], AF.Sqrt)
                for n, stage in enumerate(stages):
                    lit_tail(t, n, stage)
                nc.sync.dma_start(xl_state_out[t, :, :], xlT[:])

    nc.finalize()
    return nc


def _prep_inputs(edge_clause, edge_lit, x_l0, x_c0,
                 W_ih_lc, W_hh_lc, b_ih_lc, b_hh_lc,
                 W_ih_cl, W_hh_cl, b_ih_cl, b_hh_cl, true_vec_w):
    bf = ml_dtypes.bfloat16
    f8 = ml_dtypes.float8_e4m3fn
    xl_n = x_l0 / np.linalg.norm(x_l0, axis=1, keepdims=True)
    xc_n = x_c0 / np.linalg.norm(x_c0, axis=1, keepdims=True)

    lit_local = (edge_lit % LITS_PER).astype(np.int64)
    A = np.zeros((NC, LITS_PER), np.float32)
    np.add.at(A, (edge_clause.astype(np.int64), lit_local), 1.0)

    in_maps = []
    shared = {
        "w_yl": W_ih_lc.T.copy().astype(bf),
        "whh_lc_T": W_hh_lc.T.copy().astype(bf),
        "wa_T": W_ih_cl[:, :D].T.copy().astype(bf),
        "wb_T": W_ih_cl[:, D:].T.copy().astype(bf),
        "whh_cl_T": W_hh_cl.T.copy().astype(bf),
        "bias_lc": (b_ih_lc + b_hh_lc).reshape(D, 1).astype(np.float32),
        "bias_cl": (b_ih_cl + b_hh_cl).reshape(D, 1).astype(np.float32),
        "ones_m": np.ones((D, D), np.float32),
    }
    for m in range(M):
        lit0, cl0 = m * NLc, m * NCc
        xlT = np.ascontiguousarray(xl_n[lit0:lit0 + NLc].T).astype(bf)
        xcT = np.ascontiguousarray(xc_n[cl0:cl0 + NCc].T).astype(bf)
        a_lc = np.empty((D, GPC * 4 * CLS_PER), np.float32)
        a_cl = np.empty((D, GPC * 8 * LITS_PER), np.float32)
        for g in range(GPC):
            Ag = A[cl0 + g * CLS_PER: cl0 + (g + 1) * CLS_PER]       # [1024, 512]
            AgT = Ag.T                                                # [512, 1024]
            for kt in range(4):
                a_lc[:, (g * 4 + kt) * CLS_PER:(g * 4 + kt + 1) * CLS_PER] = \
                    AgT[kt * D:(kt + 1) * D]
            for kt in range(8):
                a_cl[:, (g * 8 + kt) * LITS_PER:(g * 8 + kt + 1) * LITS_PER] = \
                    Ag[kt * D:(kt + 1) * D]
        in_maps.append({
            "xl0T": xlT, "xc0T": xcT,
            "adj_lc": a_lc.astype(f8), "adj_cl": a_cl.astype(f8),
            **shared,
        })
    return in_maps, xl_n, xc_n


def kernel(edge_clause, edge_lit, x_l0, x_c0,
           W_ih_lc, W_hh_lc, b_ih_lc, b_hh_lc,
           W_ih_cl, W_hh_cl, b_ih_cl, b_hh_cl,
           L_vote_w, L_vote_b, true_vec_w, num_iters, **kw):
    edge_clause = np.asarray(edge_clause)
    edge_lit = np.asarray(edge_lit)
    x_l0 = np.asarray(x_l0, np.float32)
    x_c0 = np.asarray(x_c0, np.float32)
    W_ih_lc = np.asarray(W_ih_lc, np.float32)
    W_hh_lc = np.asarray(W_hh_lc, np.float32)
    b_ih_lc = np.asarray(b_ih_lc, np.float32)
    b_hh_lc = np.asarray(b_hh_lc, np.float32)
    W_ih_cl = np.asarray(W_ih_cl, np.float32)
    W_hh_cl = np.asarray(W_hh_cl, np.float32)
    b_ih_cl = np.asarray(b_ih_cl, np.float32)
    b_hh_cl = np.asarray(b_hh_cl, np.float32)
    L_vote_w = np.asarray(L_vote_w, np.float32)
    L_vote_b = np.asarray(L_vote_b, np.float32)
    true_vec_w = np.asarray(true_vec_w, np.float32)
    assert int(np.asarray(num_iters)) == T

    in_maps, xl_n0, xc_n0 = _prep_inputs(
        edge_clause, edge_lit, x_l0, x_c0, W_ih_lc, W_hh_lc, b_ih_lc, b_hh_lc,
        W_ih_cl, W_hh_cl, b_ih_cl, b_hh_cl, true_vec_w)

    if "nc" not in _CACHE:
        _CACHE["nc"] = _build_kernel()
    res = run_bass_kernel_spmd(_CACHE["nc"], in_maps, core_ids=list(range(M)),
                               trace=PROFILE)
    global LAST_RESULTS
    LAST_RESULTS = res

    # ---- host-side assembly ----
    truth_all = np.empty((T + 1, NL, 1), np.float32)
    clause_all = np.empty((T + 1, NC, D), np.float32)
    x_l = np.empty((NL, D), np.float32)
    truth_all[0] = (xl_n0 @ true_vec_w.T).reshape(NL, 1)
    clause_all[0] = xc_n0
    tvec = true_vec_w.reshape(D).astype(np.float32)
    for m in range(M):
        r = res.results[m]
        lit0, cl0 = m * NLc, m * NCc
        # truth from exported bf16 literal states
        xs = r["xl_state_out"].astype(np.float32)          # [T, D, NLc]
        truth_all[1:, lit0:lit0 + NLc, 0] = np.einsum("tdn,d->tn", xs, tvec)
        inv = np.sqrt(r["rcp_c_out"])                       # 1/norm per clause col
        snap = r["clause_out"].astype(np.float32) * inv[:, None, :]
        clause_all[1:, cl0:cl0 + NCc, :] = snap.transpose(0, 2, 1)
        x_l[lit0:lit0 + NLc] = r["xl_fin_out"].T

    x_l_vote = x_l @ L_vote_w.T + L_vote_b
    vote_mean_pool = x_l_vote.reshape(B, LITS_PER, 1).mean(axis=1)
    return (x_l_vote, x_l, vote_mean_pool, truth_all[-1], truth_all,
            clause_all, truth_all[0])


# revision 14
# speedup vs baseline: 1.9925x; 1.0093x over previous
"""NeuroSAT message-passing RNN on 8 Trainium2 NeuronCores.

Per core (8 graphs: 4096 literals, 8192 clauses), all state resident in SBUF
in TRANSPOSED layout [D=128 partitions, nodes], states in bf16:
- Sparse clause<->literal segment-sums run on the TensorEngine as block-dense
  count matmuls against per-graph adjacency blocks in fp8 (counts exact),
  fused with the RNN input projections:
    LC:  pre_c = (W_ih_lc @ x_l^T) @ A^T + W_hh_lc @ x_c^T
    CL:  pre_l = (W_A @ x_cnew^T) @ A + W_B @ flip(x_l)^T + W_hh_cl @ x_l^T
  where y_l = x_l @ W_ih_lc^T and z_c = x_cnew @ W_A^T are materialized in
  normal layout via N=128 matmuls (lhsT = transposed-state slices).
- tanh+bias on ScalarE (PSUM -> bf16); per-column sum-of-squares via an
  all-ones [128,128] f32r matmul producing the partition-broadcast result;
  1/sumsq via DVE reciprocal_approx_fast; 1/norm via one batched ACT Sqrt per
  phase (avoids ACT table thrash); squares on GpSimd.
- Clause snapshots leave unnormalized in bf16 + per-column 1/sumsq rows; the
  host normalizes. Literal states are exported per iteration (bf16) and truth
  projections computed on host; final x_l is normalized on-chip in fp32.
"""

import numpy as np
import ml_dtypes

import concourse.bass as bass
import concourse.tile as tile
from concourse import bacc, mybir
from concourse.bass import ts
from concourse.bass_utils import run_bass_kernel_spmd

F32 = mybir.dt.float32
F32R = mybir.dt.float32r
BF16 = mybir.dt.bfloat16
FP8 = mybir.dt.float8e4
AF = mybir.ActivationFunctionType
MUL = mybir.AluOpType.mult

B = 64
LITS_PER = 512
CLS_PER = 1024
NL = B * LITS_PER
NC = B * CLS_PER
D = 128
T = 8
M = 8                    # cores
GPC = B // M             # graphs per core
NLc = GPC * LITS_PER     # 4096
NCc = GPC * CLS_PER      # 8192
W = 1024                 # working tile width

_CACHE = {}
PROFILE = False
LAST_RESULTS = None


def _build_kernel():
    nc = bacc.Bacc(None, target_bir_lowering=False)

    # ---- DRAM inputs ----
    xl0T = nc.dram_tensor("xl0T", [D, NLc], BF16, kind="ExternalInput")
    xc0T = nc.dram_tensor("xc0T", [D, NCc], BF16, kind="ExternalInput")
    adj_lc = nc.dram_tensor("adj_lc", [D, GPC * 4 * CLS_PER], FP8, kind="ExternalInput")
    adj_cl = nc.dram_tensor("adj_cl", [D, GPC * 8 * LITS_PER], FP8, kind="ExternalInput")
    w_yl = nc.dram_tensor("w_yl", [D, D], BF16, kind="ExternalInput")       # W_ih_lc^T (rhs)
    whh_lc_T = nc.dram_tensor("whh_lc_T", [D, D], BF16, kind="ExternalInput")
    wa_T = nc.dram_tensor("wa_T", [D, D], BF16, kind="ExternalInput")       # W_ih_cl[:, :D]^T (rhs)
    wb_T = nc.dram_tensor("wb_T", [D, D], BF16, kind="ExternalInput")       # (W_ih_cl[:, D:])^T (lhsT)
    whh_cl_T = nc.dram_tensor("whh_cl_T", [D, D], BF16, kind="ExternalInput")
    bias_lc_in = nc.dram_tensor("bias_lc", [D, 1], F32, kind="ExternalInput")
    bias_cl_in = nc.dram_tensor("bias_cl", [D, 1], F32, kind="ExternalInput")
    ones_m_in = nc.dram_tensor("ones_m", [D, D], F32R, kind="ExternalInput")

    # ---- DRAM outputs ----
    clause_out = nc.dram_tensor("clause_out", [T, D, NCc], BF16, kind="ExternalOutput")
    rcp_c_out = nc.dram_tensor("rcp_c_out", [T, NCc], F32, kind="ExternalOutput")
    xl_state_out = nc.dram_tensor("xl_state_out", [T, D, NLc], BF16, kind="ExternalOutput")
    xl_fin_out = nc.dram_tensor("xl_fin_out", [D, NLc], F32, kind="ExternalOutput")

    with tile.TileContext(nc) as tc:
        with (
            tc.tile_pool(name="cst", bufs=1) as cst,
            tc.tile_pool(name="st", bufs=1) as st,
            tc.tile_pool(name="stg", bufs=6) as stg,
            tc.tile_pool(name="wk", bufs=3) as wk,
            tc.tile_pool(name="pmm", bufs=2, space="PSUM") as pmm,
            tc.tile_pool(name="psq", bufs=2, space="PSUM") as psq,
            tc.tile_pool(name="pzy", bufs=1, space="PSUM") as pzy,
        ):
            # persistent tensors
            xlT = st.tile([D, NLc], BF16, tag="xlT")
            xcT = st.tile([D, NCc], BF16, tag="xcT")
            y_l = st.tile([D, NLc], BF16, tag="y_l")       # 32 normal-layout tiles
            z_c = st.tile([D, NCc], BF16, tag="z_c")       # 64 normal-layout tiles
            rcp = st.tile([D, NLc], F32, tag="rcp")        # 1/sumsq, bcast rows
            inv = st.tile([D, NLc], BF16, tag="inv")       # 1/norm, bcast rows
            a_lc = cst.tile([D, GPC * 4 * CLS_PER], FP8, tag="a_lc")
            a_cl = cst.tile([D, GPC * 8 * LITS_PER], FP8, tag="a_cl")
            wyl = cst.tile([D, D], BF16, tag="wyl")
            whhlc = cst.tile([D, D], BF16, tag="whhlc")
            waT = cst.tile([D, D], BF16, tag="waT")
            wbT = cst.tile([D, D], BF16, tag="wbT")
            whhcl = cst.tile([D, D], BF16, tag="whhcl")
            b_lc = cst.tile([D, 1], F32, tag="b_lc")
            b_cl = cst.tile([D, 1], F32, tag="b_cl")
            ones_m = cst.tile([D, D], F32R, tag="ones_m")

            nc.gpsimd.dma_start(xlT[:], xl0T[:])
            nc.gpsimd.dma_start(xcT[:], xc0T[:])
            nc.gpsimd.dma_start(a_lc[:], adj_lc[:])
            nc.gpsimd.dma_start(a_cl[:], adj_cl[:])
            nc.gpsimd.dma_start(wyl[:], w_yl[:])
            nc.gpsimd.dma_start(whhlc[:], whh_lc_T[:])
            nc.gpsimd.dma_start(waT[:], wa_T[:])
            nc.gpsimd.dma_start(wbT[:], wb_T[:])
            nc.gpsimd.dma_start(whhcl[:], whh_cl_T[:])
            nc.gpsimd.dma_start(b_lc[:], bias_lc_in[:])
            nc.gpsimd.dma_start(b_cl[:], bias_cl_in[:])
            nc.gpsimd.dma_start(ones_m[:], ones_m_in[:])

            def norm_head(stage, off):
                """sq -> sumsq (bcast matmul) -> reciprocal into rcp[:, off:off+W]."""
                sq = wk.tile([D, W], F32R, tag="sq")
                nc.gpsimd.tensor_tensor(sq[:], stage[:], stage[:], op=MUL)
                for h in range(2):
                    pq = psq.tile([D, 512], F32, tag="sq")
                    nc.tensor.matmul(pq[:], ones_m[:], sq[:, ts(h, 512)], start=True, stop=True)
                    nc.vector.reciprocal_approx_fast(rcp[:, off + h * 512:off + h * 512 + 512], pq[:])

            def clause_head(t, n):
                """n-th [128, 1024] clause tile (= graph n): matmuls + tanh + norm.

                MMs for the two 512-halves are interleaved so LDWEIGHTS of one
                half overlaps the other half's stream; hh comes first (no dep
                on y_l).
                """
                ps = pmm.tile([D, W], F32, tag="mm")
                sls = [slice(0, 512), slice(512, W)]
                for h in range(2):
                    nc.tensor.matmul(ps[:, sls[h]], whhlc[:],
                                     xcT[:, n * W + h * 512:][:, :512],
                                     start=True, stop=False)
                for kt in range(4):
                    for h in range(2):
                        nc.tensor.matmul(
                            ps[:, sls[h]],
                            y_l[:, ts(n * 4 + kt, D)],
                            a_lc[:, n * 4 * CLS_PER + kt * CLS_PER + h * 512:][:, :512],
                            start=False,
                            stop=(kt == 3),
                        )
                stage = stg.tile([D, W], BF16, tag="stage")
                nc.scalar.activation(stage[:], ps[:], AF.Tanh, bias=b_lc[:])
                nc.sync.dma_start(clause_out[t, :, ts(n, W)], stage[:])
                norm_head(stage, (n % 4) * W)
                return stage

            def clause_tail(t, n, stage):
                pz = pzy.tile([D, W], F32, tag="zy")
                for q in range(8):
                    nc.tensor.matmul(pz[:, ts(q, D)], stage[:, ts(q, D)], waT[:],
                                     start=True, stop=True)
                nc.vector.tensor_copy(z_c[:, ts(n, W)], pz[:])
                nc.vector.tensor_mul(xcT[:, ts(n, W)], stage[:], inv[:, ts(n % 4, W)])

            def lit_head(t, n):
                """n-th [128, 1024] literal tile covers graphs 2n, 2n+1."""
                ps = pmm.tile([D, W], F32, tag="mm")
                sls = [slice(0, 512), slice(512, W)]
                for h in range(2):
                    o = (2 * n + h) * LITS_PER
                    nc.tensor.matmul(ps[:, sls[h]], whhcl[:], xlT[:, o:o + 512],
                                     start=True, stop=False)
                for h in range(2):
                    o = (2 * n + h) * LITS_PER
                    nc.tensor.matmul(ps[:, h * 512:h * 512 + 256], wbT[:],
                                     xlT[:, o + 256:o + 512], start=False, stop=False)
                    nc.tensor.matmul(ps[:, h * 512 + 256:h * 512 + 512], wbT[:],
                                     xlT[:, o:o + 256], start=False, stop=False)
                for kt in range(8):
                    for h in range(2):
                        g = 2 * n + h
                        nc.tensor.matmul(
                            ps[:, sls[h]],
                            z_c[:, ts(g * 8 + kt, D)],
                            a_cl[:, g * 8 * LITS_PER + kt * LITS_PER:][:, :512],
                            start=False,
                            stop=(kt == 7),
                        )
                stage = stg.tile([D, W], BF16, tag="stage")
                nc.scalar.activation(stage[:], ps[:], AF.Tanh, bias=b_cl[:])
                norm_head(stage, n * W)
                return stage

            def lit_tail(t, n, stage):
                nc.vector.tensor_mul(xlT[:, ts(n, W)], stage[:], inv[:, ts(n, W)])
                if t == T - 1:
                    xlf = wk.tile([D, W], F32, tag="xlf")
                    nc.vector.tensor_mul(xlf[:], stage[:], inv[:, ts(n, W)])
                    nc.sync.dma_start(xl_fin_out[:, ts(n, W)], xlf[:])
                else:
                    py = pzy.tile([D, W], F32, tag="zy")
                    for q in range(8):
                        nc.tensor.matmul(py[:, ts(q, D)], xlT[:, ts(n * 8 + q, D)],
                                         wyl[:], start=True, stop=True)
                    nc.scalar.activation(y_l[:, ts(n, W)], py[:], AF.Copy)

            for t in range(T):
                if t == 0:
                    # ---- y_l = x_l @ W_ih_lc^T (normal layout) ----
                    for j8 in range(NLc // W):
                        py = pzy.tile([D, W], F32, tag="zy")
                        for q in range(8):
                            nc.tensor.matmul(py[:, ts(q, D)], xlT[:, ts(j8 * 8 + q, D)],
                                             wyl[:], start=True, stop=True)
                        nc.scalar.activation(y_l[:, ts(j8, W)], py[:], AF.Copy)

                # ---- clause update: 2 half-batches of 4 tiles ----
                for half in range(2):
                    ns = range(half * 4, half * 4 + 4)
                    stages = [clause_head(t, n) for n in ns]
                    nc.scalar.activation(inv[:], rcp[:], AF.Sqrt)
                    nc.sync.dma_start(rcp_c_out[t:t + 1, ts(half, NLc)], rcp[:1, :])
                    for n, stage in zip(ns, stages):
                        clause_tail(t, n, stage)

                # ---- literal update: 4 heads, one batched sqrt, 4 tails ----
                stages = [lit_head(t, n) for n in range(NLc // W)]
                nc.scalar.activation(inv[:, :NLc], rcp[:, :NLc], AF.Sqrt)
                for n, stage in enumerate(stages):
                    lit_tail(t, n, stage)
                nc.sync.dma_start(xl_state_out[t, :, :], xlT[:])

    nc.finalize()
    return nc


def _prep_inputs(edge_clause, edge_lit, x_l0, x_c0,
                 W_ih_lc, W_hh_lc, b_ih_lc, b_hh_lc,
                 W_ih_cl, W_hh_cl, b_ih_cl, b_hh_cl, true_vec_w):
    bf = ml_dtypes.bfloat16
    f8 = ml_dtypes.float8_e4m3fn
    xl_n = x_l0 / np.linalg.norm(x_l0, axis=1, keepdims=True)
    xc_n = x_c0 / np.linalg.norm(x_c0, axis=1, keepdims=True)

    lit_local = (edge_lit % LITS_PER).astype(np.int64)
    A = np.zeros((NC, LITS_PER), np.float32)
    np.add.at(A, (edge_clause.astype(np.int64), lit_local), 1.0)

    in_maps = []
    shared = {
        "w_yl": W_ih_lc.T.copy().astype(bf),
        "whh_lc_T": W_hh_lc.T.copy().astype(bf),
        "wa_T": W_ih_cl[:, :D].T.copy().astype(bf),
        "wb_T": W_ih_cl[:, D:].T.copy().astype(bf),
        "whh_cl_T": W_hh_cl.T.copy().astype(bf),
        "bias_lc": (b_ih_lc + b_hh_lc).reshape(D, 1).astype(np.float32),
        "bias_cl": (b_ih_cl + b_hh_cl).reshape(D, 1).astype(np.float32),
        "ones_m": np.ones((D, D), np.float32),
    }
    for m in range(M):
        lit0, cl0 = m * NLc, m * NCc
        xlT = np.ascontiguousarray(xl_n[lit0:lit0 + NLc].T).astype(bf)
        xcT = np.ascontiguousarray(xc_n[cl0:cl0 + NCc].T).astype(bf)
        a_lc = np.empty((D, GPC * 4 * CLS_PER), np.float32)
        a_cl = np.empty((D, GPC * 8 * LITS_PER), np.float32)
        for g in range(GPC):
            Ag = A[cl0 + g * CLS_PER: cl0 + (g + 1) * CLS_PER]       # [1024, 512]
            AgT = Ag.T                                                # [512, 1024]
            for kt in range(4):
                a_lc[:, (g * 4 + kt) * CLS_PER:(g * 4 + kt + 1) * CLS_PER] = \
                    AgT[kt * D:(kt + 1) * D]
            for kt in range(8):
                a_cl[:, (g * 8 + kt) * LITS_PER:(g * 8 + kt + 1) * LITS_PER] = \
                    Ag[kt * D:(kt + 1) * D]
        in_maps.append({
            "xl0T": xlT, "xc0T": xcT,
            "adj_lc": a_lc.astype(f8), "adj_cl": a_cl.astype(f8),
            **shared,
        })
    return in_maps, xl_n, xc_n


def kernel(edge_clause, edge_lit, x_l0, x_c0,
           W_ih_lc, W_hh_lc, b_ih_lc, b_hh_lc,
           W_ih_cl, W_hh_cl, b_ih_cl, b_hh_cl,
           L_vote_w, L_vote_b, true_vec_w, num_iters, **kw):
    edge_clause = np.asarray(edge_clause)
    edge_lit = np.asarray(edge_lit)
    x_l0 = np.asarray(x_l0, np.float32)
    x_c0 = np.asarray(x_c0, np.float32)
    W_ih_lc = np.asarray(W_ih_lc, np.float32)
    W_hh_lc = np.asarray(W_hh_lc, np.float32)
    b_ih_lc = np.asarray(b_ih_lc, np.float32)
    b_hh_lc = np.asarray(b_hh_lc, np.float32)
    W_ih_cl = np.asarray(W_ih_cl, np.float32)
    W_hh_cl = np.asarray(W_hh_cl, np.float32)
    b_ih_cl = np.asarray(b_ih_cl, np.float32)
    b_hh_cl = np.asarray(b_hh_cl, np.float32)
    L_vote_w = np.asarray(L_vote_w, np.float32)
    L_vote_b = np.asarray(L_vote_b, np.float32)
    true_vec_w = np.asarray(true_vec_w, np.float32)
    assert int(np.asarray(num_iters)) == T

    in_maps, xl_n0, xc_n0 = _prep_inputs(
        edge_clause, edge_lit, x_l0, x_c0, W_ih_lc, W_hh_lc, b_ih_lc, b_hh_lc,
        W_ih_cl, W_hh_cl, b_ih_cl, b_hh_cl, true_vec_w)

    if "nc" not in _CACHE:
        _CACHE["nc"] = _build_kernel()
    res = run_bass_kernel_spmd(_CACHE["nc"], in_maps, core_ids=list(range(M)),
                               trace=PROFILE)
    global LAST_RESULTS
    LAST_RESULTS = res

    # ---- host-side assembly ----
    truth_all = np.empty((T + 1, NL, 1), np.float32)
    clause_all = np.empty((T + 1, NC, D), np.float32)
    x_l = np.empty((NL, D), np.float32)
    truth_all[0] = (xl_n0 @ true_vec_w.T).reshape(NL, 1)
    clause_all[0] = xc_n0
    tvec = true_vec_w.reshape(D).astype(np.float32)
    for m in range(M):
        r = res.results[m]
        lit0, cl0 = m * NLc, m * NCc
        # truth from exported bf16 literal states
        xs = r["xl_state_out"].astype(np.float32)          # [T, D, NLc]
        truth_all[1:, lit0:lit0 + NLc, 0] = np.einsum("tdn,d->tn", xs, tvec)
        inv = np.sqrt(r["rcp_c_out"])                       # 1/norm per clause col
        snap = r["clause_out"].astype(np.float32) * inv[:, None, :]
        clause_all[1:, cl0:cl0 + NCc, :] = snap.transpose(0, 2, 1)
        x_l[lit0:lit0 + NLc] = r["xl_fin_out"].T

    x_l_vote = x_l @ L_vote_w.T + L_vote_b
    vote_mean_pool = x_l_vote.reshape(B, LITS_PER, 1).mean(axis=1)
    return (x_l_vote, x_l, vote_mean_pool, truth_all[-1], truth_all,
            clause_all, truth_all[0])
